# revision 43
# baseline (speedup 1.0000x reference)
"""Block-sparse linear y = x @ W^T + b on 8 TRN2 NeuronCores.

Problem shape (hardcoded): x [8192, 4096] f32, weight [1024, 64, 64] f32
(64x64 blocks), bias [4096] f32, row_idx/col_idx [1024] int32 over a 64x64
block grid.

V5 strategy: data-parallel over tokens (1024/core). y^T = W x^T + b via
64x64 block matmuls packed 4-wide into the PE-array quadrants
(tile_position), bf16 inputs with f32 PSUM accumulate. x^T resident in
SBUF as ONE full copy (block c on partition half c%2, so block (r,c)
defaults to array row-group ki=c%2) plus a SMALL set of shifted "b" tiles
(128 rows at odd 64-row offsets) that let selected blocks run on the
opposite row-group — just enough to balance every pair's four quadrant
chains to equal length. Chains consume col-blocks in ascending c so
compute overlaps the x stream-in; weight groups stream on the scalar
queue so they don't serialize behind x-chunk issue. Output y^T stored
bf16, bias added on eviction, f32 cast on host.
"""

from contextlib import ExitStack

import numpy as np
import ml_dtypes

import concourse.tile as tile
from concourse import bacc, mybir
from concourse.bass_utils import run_bass_kernel_spmd

BLK = 64
OUT_BLK = 64
IN_BLK = 64
D_IN = IN_BLK * BLK    # 4096
D_OUT = OUT_BLK * BLK  # 4096
N_CORES = 8
WGRP = 32              # weight slots per DMA group (group = [128, WGRP*64])
XCH = 4                # x tiles (128-row groups) per DMA chunk
BF16 = ml_dtypes.bfloat16


def _btile_of(c):
    """Shifted tile serving block c on the opposite row-group, or -1.
    b-tile t covers x^T rows (2t+1)*64 .. (2t+3)*64: block 2t+1 on
    partitions 0-63 (ki0), block 2t+2 on partitions 64-127 (ki1)."""
    t = (c - 2) // 2 if c % 2 == 0 else (c - 1) // 2
    return t if 0 <= t <= (IN_BLK - 3) // 2 else -1


def _make_schedule(row_idx, col_idx):
    """Returns (pairs, btiles). pairs[p] = (merged, singles):
    merged = [(b, mi, w_even, w_odd)] -> one 128-row matmul per item
    (both blocks 2b, 2b+1 of row mi live in x tile b, partition halves
    0/1); singles = [(c, mi, w, ki)] with the four (ki, mi) leftover
    chains balanced via moves onto b-tiles."""
    d = {}
    for i in range(len(row_idx)):
        d[(int(row_idx[i]), int(col_idx[i]))] = i
    blocks_by_r = [[] for _ in range(OUT_BLK)]
    for (r, c), w in d.items():
        blocks_by_r[r].append((c, w))
    for lst in blocks_by_r:
        lst.sort()

    # greedy merge: rows holding both blocks of an x tile b get a single
    # full-height matmul for the pair
    merged_by_r = [[] for _ in range(OUT_BLK)]
    left_by_r = [[] for _ in range(OUT_BLK)]
    for r in range(OUT_BLK):
        cmap = dict(blocks_by_r[r])
        for b in range(IN_BLK // 2):
            if 2 * b in cmap and 2 * b + 1 in cmap:
                merged_by_r[r].append((b, cmap.pop(2 * b),
                                       cmap.pop(2 * b + 1)))
        left_by_r[r] = sorted(cmap.items())

    # decide moved singles: per row, shift surplus parity to the other ki
    from collections import Counter
    demand = Counter()
    for r in range(OUT_BLK):
        cs = [c for c, _ in left_by_r[r]]
        e = [c for c in cs if c % 2 == 0]
        o = [c for c in cs if c % 2 == 1]
        surplus = e if len(e) > len(o) else o
        for c in surplus:
            t = _btile_of(c)
            if t >= 0:
                demand[t] += 1

    S = set()
    moved = set()  # (r, c) singles running on ki = 1 - c%2
    for r in range(OUT_BLK):
        cs = [c for c, _ in left_by_r[r]]
        e = [c for c in cs if c % 2 == 0]
        o = [c for c in cs if c % 2 == 1]
        k = (len(e) - len(o)) // 2 if len(e) > len(o) else (
            len(o) - len(e)) // 2
        if k <= 0:
            continue
        cand = [(c, _btile_of(c)) for c in (e if len(e) > len(o) else o)]
        cand = [(c, t) for c, t in cand if t >= 0]
        cand.sort(key=lambda ct: (ct[1] not in S, -demand[ct[1]]))
        for c, t in cand[:k]:
            moved.add((r, c))
            S.add(t)

    pairs = []
    for p in range(OUT_BLK // 2):
        r1, r2 = 2 * p, 2 * p + 1
        merged = []
        singles = []
        for mi, r in ((0, r1), (1, r2)):
            for (b, we, wo) in merged_by_r[r]:
                merged.append((b, mi, we, wo))
            for c, w in left_by_r[r]:
                ki = (1 - c % 2) if (r, c) in moved else (c % 2)
                singles.append((c, mi, w, ki))
        singles.sort()
        # alternate mi among merged items, ascending b within each mi
        m0 = sorted(m for m in merged if m[1] == 0)
        m1 = sorted(m for m in merged if m[1] == 1)
        woven_m = []
        while m0 or m1:
            if m0:
                woven_m.append(m0.pop(0))
            if m1:
                woven_m.append(m1.pop(0))
        pairs.append((woven_m, singles))
    return pairs, sorted(S)


def _assign_slots(pairs):
    """Weight slot per item in emission order. Merged items use a full
    slot (both ki halves); singles fill half-slots, pairing opposite-ki
    neighbors into one slot."""
    slot = {}
    nxt = 0
    for p, (merged, singles) in enumerate(pairs):
        for (b, mi, we, wo) in merged:
            slot[(p, "m", b, mi)] = nxt
            nxt += 1
        open_slot = [None, None]  # slot index with free half per ki
        for (c, mi, w, ki) in _weave(singles):
            if open_slot[ki] is not None:
                slot[(p, c, mi)] = open_slot[ki]
                open_slot[ki] = None
            else:
                slot[(p, c, mi)] = nxt
                open_slot[1 - ki] = nxt
                nxt += 1
    n_groups = (nxt + WGRP - 1) // WGRP
    return slot, max(n_groups, 1)


def _weave(blocks):
    """Interleave the four (ki, mi) chains, rotating quadrants."""
    chains = {(ki, mi): [] for ki in (0, 1) for mi in (0, 1)}
    for b in blocks:
        c, mi, w, ki = b
        chains[(ki, mi)].append(b)
    order = [(0, 0), (1, 1), (0, 1), (1, 0)]
    woven = []
    for q in chains:
        # moved (b-tile) blocks last so late b-tile arrival never gates
        # the front of a chain
        chains[q].sort(key=lambda b: (b[3] != b[0] % 2, b[0]))
    while any(chains.values()):
        for q in order:
            if chains[q]:
                woven.append(chains[q].pop(0))
    return woven


def _pack_host_arrays(weight, bias, pairs, slot, n_groups):
    wgrp = np.zeros((n_groups, 128, WGRP * BLK), dtype=BF16)
    bias_pk = np.zeros((128, len(pairs)), dtype=np.float32)
    wT = np.ascontiguousarray(
        np.transpose(np.asarray(weight), (0, 2, 1))).astype(BF16)
    for p, (merged, singles) in enumerate(pairs):
        r1, r2 = 2 * p, 2 * p + 1
        bias_pk[:64, p] = bias[r1 * BLK:(r1 + 1) * BLK]
        bias_pk[64:, p] = bias[r2 * BLK:(r2 + 1) * BLK]
        for (b, mi, we, wo) in merged:
            g, j = divmod(slot[(p, "m", b, mi)], WGRP)
            wgrp[g, 0:64, j * BLK:(j + 1) * BLK] = wT[we]
            wgrp[g, 64:128, j * BLK:(j + 1) * BLK] = wT[wo]
        for (c, mi, w, ki) in singles:
            g, j = divmod(slot[(p, c, mi)], WGRP)
            wgrp[g, ki * 64:(ki + 1) * 64, j * BLK:(j + 1) * BLK] = wT[w]
    return wgrp, bias_pk


def _build_kernel(pairs, slot, n_groups, btiles, ntok,
                  w_bufs=5, ps_bufs=8, out_bufs=6):
    assert ntok % 512 == 0
    n_th = ntok // 512
    sdt = mybir.dt.bfloat16
    f32 = mybir.dt.float32

    nc = bacc.Bacc("TRN2", target_bir_lowering=False, debug=False)
    n_chunks = (IN_BLK // 2 + XCH - 1) // XCH
    xt_d = nc.dram_tensor("xt_pk", [n_chunks, 128, XCH * ntok], sdt,
                          kind="ExternalInput").ap()
    n_b = max(len(btiles), 1)
    xb_d = nc.dram_tensor("xb_pk", [n_b, 128, ntok], sdt,
                          kind="ExternalInput").ap()
    wg_d = nc.dram_tensor("wgrp", [n_groups, 128, WGRP * BLK], sdt,
                          kind="ExternalInput").ap()
    bias_d = nc.dram_tensor("bias_pk", [128, len(pairs)], f32,
                            kind="ExternalInput").ap()
    yt_d = nc.dram_tensor("yt", [D_OUT, ntok], sdt,
                          kind="ExternalOutput").ap()
    b_index = {tb: k for k, tb in enumerate(btiles)}

    with tile.TileContext(nc) as tc:
        with ExitStack() as ctx:
            xpool = ctx.enter_context(tc.tile_pool(name="xp", bufs=1))
            wpool = ctx.enter_context(tc.tile_pool(name="wp", bufs=w_bufs))
            pspool = ctx.enter_context(
                tc.tile_pool(name="ps", bufs=ps_bufs, space="PSUM"))
            opool = ctx.enter_context(tc.tile_pool(name="op", bufs=out_bufs))
            bpool = ctx.enter_context(tc.tile_pool(name="bp", bufs=1))

            bias_sb = bpool.tile([128, len(pairs)], f32, tag="bias",
                                 name="bias_sb")
            nc.scalar.dma_start(bias_sb[:], bias_d[:])

            xchunks = {}
            xbt = {}

            def x_chunk(cb):
                if cb not in xchunks:
                    t = xpool.tile([128, XCH * ntok], sdt, tag=f"x{cb}",
                                   name=f"x{cb}")
                    nc.sync.dma_start(t[:], xt_d[cb, :, :])
                    xchunks[cb] = t
                return xchunks[cb]

            def x_btile(tb):
                if tb not in xbt:
                    t = xpool.tile([128, ntok], sdt, tag=f"xb{tb}",
                                   name=f"xb{tb}")
                    nc.sync.dma_start(t[:], xb_d[b_index[tb], :, :])
                    xbt[tb] = t
                return xbt[tb]

            def x_ap(c, ki, th):
                if ki == c % 2:
                    b = c // 2
                    cb, wi = divmod(b, XCH)
                    t = x_chunk(cb)
                    o = wi * ntok + th * 512
                else:
                    t = x_btile(_btile_of(c))
                    o = th * 512
                return t[ki * 64:(ki + 1) * 64, o:o + 512]

            # eager-issue all x chunks ahead of everything else
            for cb in range(n_chunks):
                x_chunk(cb)

            # region totals per psum tile half: merged items accumulate
            # into tile 0 alongside the ki0 singles chain
            nmm = []
            for (merged, singles) in pairs:
                m = {(ki, mi): 0 for ki in (0, 1) for mi in (0, 1)}
                for (b, mi, we, wo) in merged:
                    m[(0, mi)] += 1
                for (c, mi, w, ki) in singles:
                    m[(ki, mi)] += 1
                nmm.append(m)

            psum = {}
            wg_tiles = {}

            def ensure_psum(p, th):
                if (p, th) not in psum:
                    psum[(p, th)] = [
                        pspool.tile([128, 512], f32, tag="ps",
                                    name=f"ps{p}_{th}_{k}") for k in range(2)]

            def ensure_wgroup(g):
                for gpf in (g, g + 1, g + 2):
                    if gpf < n_groups and gpf not in wg_tiles:
                        wg_tiles[gpf] = wpool.tile(
                            [128, WGRP * BLK], sdt, tag="wg",
                            name=f"wg{gpf}")
                        nc.scalar.dma_start(wg_tiles[gpf][:],
                                            wg_d[gpf, :, :])

            osb_cur = {}

            def eviction_th(p, th):
                if p not in osb_cur:
                    osb_cur[p] = opool.tile([128, n_th * 512], sdt,
                                            tag="o16", name=f"o{p}")
                osb = osb_cur[p][:, th * 512:(th + 1) * 512]
                pt = psum.pop((p, th))
                if all(v > 0 for v in nmm[p].values()):
                    nc.scalar.activation(
                        osb, pt[0][:],
                        mybir.ActivationFunctionType.Identity,
                        bias=bias_sb[:, p:p + 1], scale=1.0)
                    nc.vector.tensor_add(osb, osb, pt[1][:])
                else:
                    for mi in (0, 1):
                        oh = osb[mi * 64:(mi + 1) * 64, :]
                        bh = bias_sb[mi * 64:(mi + 1) * 64, p:p + 1]
                        srcs = [pt[ki][mi * 64:(mi + 1) * 64, :]
                                for ki in (0, 1) if nmm[p][(ki, mi)] > 0]
                        if not srcs:
                            nc.vector.memset(oh, 0.0)
                            nc.vector.tensor_scalar_add(oh, oh, bh)
                        else:
                            nc.scalar.activation(
                                oh, srcs[0],
                                mybir.ActivationFunctionType.Identity,
                                bias=bh, scale=1.0)
                            if len(srcs) > 1:
                                nc.vector.tensor_add(oh, oh, srcs[1])
                if th == n_th - 1:
                    nc.sync.dma_start(yt_d[p * 128:(p + 1) * 128, :],
                                      osb_cur.pop(p)[:])

            for p, (merged, singles) in enumerate(pairs):
                if not merged and not singles:
                    osb = opool.tile([128, n_th * 512], sdt, tag="o16",
                                     name=f"oz{p}")
                    nc.vector.memset(osb[:], 0.0)
                    nc.vector.tensor_scalar_add(osb[:], osb[:],
                                                bias_sb[:, p:p + 1])
                    nc.sync.dma_start(yt_d[p * 128:(p + 1) * 128, :], osb[:])
                    continue
                for th in range(n_th):
                    ensure_psum(p, th)
                done = {(th, ki, mi): 0 for th in range(n_th)
                        for ki in (0, 1) for mi in (0, 1)}
                # merged items: one full-height (128-row) matmul computes
                # both blocks of x tile b; mi alternation keeps the two
                # array column-halves concurrently busy
                for (b, mi, we, wo) in merged:
                    g, j = divmod(slot[(p, "m", b, mi)], WGRP)
                    ensure_wgroup(g)
                    lhsT = wg_tiles[g][:, j * BLK:(j + 1) * BLK]
                    cb, wi = divmod(b, XCH)
                    t = x_chunk(cb)
                    for th in range(n_th):
                        done[(th, 0, mi)] += 1
                        first = done[(th, 0, mi)] == 1
                        last = done[(th, 0, mi)] == nmm[p][(0, mi)]
                        o = wi * ntok + th * 512
                        nc.tensor.matmul(
                            psum[(p, th)][0][mi * 64:(mi + 1) * 64, :],
                            lhsT, t[:, o:o + 512],
                            start=first, stop=last,
                            tile_position=(0, mi * 64),
                            skip_group_check=True,
                        )
                for (c, mi, w, ki) in _weave(singles):
                    g, j = divmod(slot[(p, c, mi)], WGRP)
                    ensure_wgroup(g)
                    lhsT = wg_tiles[g][ki * 64:(ki + 1) * 64,
                                       j * BLK:(j + 1) * BLK]
                    for th in range(n_th):
                        done[(th, ki, mi)] += 1
                        first = done[(th, ki, mi)] == 1
                        last = done[(th, ki, mi)] == nmm[p][(ki, mi)]
                        nc.tensor.matmul(
                            psum[(p, th)][ki][mi * 64:(mi + 1) * 64, :],
                            lhsT, x_ap(c, ki, th),
                            start=first, stop=last,
                            tile_position=(ki * 64, mi * 64),
                            skip_group_check=True,
                        )
                for th in range(n_th):
                    eviction_th(p, th)
    nc.compile()
    return nc


def kernel(x, weight, bias, row_idx, col_idx):
    x = np.asarray(x, dtype=np.float32)
    weight = np.asarray(weight, dtype=np.float32)
    bias = np.asarray(bias, dtype=np.float32)
    row_idx = np.asarray(row_idx)
    col_idx = np.asarray(col_idx)
    ntok_total = x.shape[0]
    assert ntok_total % N_CORES == 0
    ntok = ntok_total // N_CORES

    pairs, btiles = _make_schedule(row_idx, col_idx)
    slot, n_groups = _assign_slots(pairs)
    wgrp, bias_pk = _pack_host_arrays(weight, bias, pairs, slot, n_groups)
    nc = _build_kernel(pairs, slot, n_groups, btiles, ntok)

    n_chunks = (IN_BLK // 2 + XCH - 1) // XCH
    in_maps = []
    for c in range(N_CORES):
        xt = np.ascontiguousarray(
            x[c * ntok:(c + 1) * ntok].T).astype(BF16)
        # chunk-contiguous: [cb, partition, wi*ntok + t]
        xt_pk = np.ascontiguousarray(
            xt.reshape(n_chunks, XCH, 128, ntok).transpose(0, 2, 1, 3)
            .reshape(n_chunks, 128, XCH * ntok))
        if btiles:
            xb_pk = np.stack(
                [xt[(2 * tb + 1) * 64:(2 * tb + 1) * 64 + 128] for tb in btiles])
        else:
            xb_pk = np.zeros((1, 128, ntok), dtype=BF16)
        xb_pk = np.ascontiguousarray(xb_pk)
        in_maps.append({"xt_pk": xt_pk, "xb_pk": xb_pk, "wgrp": wgrp,
                        "bias_pk": bias_pk})

    res = run_bass_kernel_spmd(nc, in_maps, core_ids=list(range(N_CORES)))
    y = np.empty((ntok_total, D_OUT), dtype=np.float32)
    for c in range(N_CORES):
        y[c * ntok:(c + 1) * ntok] = res.results[c]["yt"].T.astype(np.float32)
    return y


# revision 44
# speedup vs baseline: 1.1859x; 1.1859x over previous
"""Block-sparse linear y = x @ W^T + b on 8 TRN2 NeuronCores.

Problem shape (hardcoded): x [8192, 4096] f32, weight [1024, 64, 64] f32
(64x64 blocks), bias [4096] f32, row_idx/col_idx [1024] int32 over a 64x64
block grid.

V5 strategy: data-parallel over tokens (1024/core). y^T = W x^T + b via
64x64 block matmuls packed 4-wide into the PE-array quadrants
(tile_position), bf16 inputs with f32 PSUM accumulate. x^T resident in
SBUF as ONE full copy (block c on partition half c%2, so block (r,c)
defaults to array row-group ki=c%2) plus a SMALL set of shifted "b" tiles
(128 rows at odd 64-row offsets) that let selected blocks run on the
opposite row-group — just enough to balance every pair's four quadrant
chains to equal length. Chains consume col-blocks in ascending c so
compute overlaps the x stream-in; weight groups stream on the scalar
queue so they don't serialize behind x-chunk issue. Output y^T stored
bf16, bias added on eviction, f32 cast on host.
"""

from contextlib import ExitStack

import numpy as np
import ml_dtypes

import concourse.tile as tile
from concourse import bacc, mybir
from concourse.bass_utils import run_bass_kernel_spmd

BLK = 64
OUT_BLK = 64
IN_BLK = 64
D_IN = IN_BLK * BLK    # 4096
D_OUT = OUT_BLK * BLK  # 4096
N_CORES = 8
WGRP = 32              # weight slots per DMA group (group = [128, WGRP*64])
XCH = 4                # x tiles (128-row groups) per DMA chunk
BF16 = ml_dtypes.bfloat16


def _btile_of(c):
    """Shifted tile serving block c on the opposite row-group, or -1.
    b-tile t covers x^T rows (2t+1)*64 .. (2t+3)*64: block 2t+1 on
    partitions 0-63 (ki0), block 2t+2 on partitions 64-127 (ki1)."""
    t = (c - 2) // 2 if c % 2 == 0 else (c - 1) // 2
    return t if 0 <= t <= (IN_BLK - 3) // 2 else -1


def _make_schedule(row_idx, col_idx):
    """Returns (pairs, btiles): pairs[p] = list of (c, mi, w, ki) with the
    four (ki, mi) chains balanced per pair via moves onto b-tiles."""
    d = {}
    for i in range(len(row_idx)):
        d[(int(row_idx[i]), int(col_idx[i]))] = i
    blocks_by_r = [[] for _ in range(OUT_BLK)]
    for (r, c), w in d.items():
        blocks_by_r[r].append((c, w))
    for lst in blocks_by_r:
        lst.sort()

    # decide moved blocks: per row, shift surplus parity to the other ki
    from collections import Counter
    demand = Counter()
    for r in range(OUT_BLK):
        cs = [c for c, _ in blocks_by_r[r]]
        e = [c for c in cs if c % 2 == 0]
        o = [c for c in cs if c % 2 == 1]
        surplus = e if len(e) > len(o) else o
        for c in surplus:
            t = _btile_of(c)
            if t >= 0:
                demand[t] += 1

    S = set()
    moved = set()  # (r, c) pairs running on ki = 1 - c%2
    for r in range(OUT_BLK):
        cs = [c for c, _ in blocks_by_r[r]]
        e = [c for c in cs if c % 2 == 0]
        o = [c for c in cs if c % 2 == 1]
        k = (len(e) - len(o)) // 2 if len(e) > len(o) else (
            len(o) - len(e)) // 2
        if k <= 0:
            continue
        cand = [(c, _btile_of(c)) for c in (e if len(e) > len(o) else o)]
        cand = [(c, t) for c, t in cand if t >= 0]
        cand.sort(key=lambda ct: (ct[1] not in S, -demand[ct[1]]))
        for c, t in cand[:k]:
            moved.add((r, c))
            S.add(t)

    pairs = []
    for p in range(OUT_BLK // 2):
        r1, r2 = 2 * p, 2 * p + 1
        blocks = []
        for mi, r in ((0, r1), (1, r2)):
            for c, w in blocks_by_r[r]:
                ki = (1 - c % 2) if (r, c) in moved else (c % 2)
                blocks.append((c, mi, w, ki))
        blocks.sort()
        pairs.append(blocks)
    return pairs, sorted(S)


def _assign_slots(pairs):
    """Weight slot per block: ki-half 0/1 fill slots independently, in
    (woven) emission order so group DMAs stream with consumption."""
    slot = {}
    cnt = [0, 0]
    for p, blocks in enumerate(pairs):
        for (c, mi, w, ki) in _weave(blocks):
            slot[(p, c, mi)] = cnt[ki]
            cnt[ki] += 1
    n_slots = max(cnt)
    n_groups = (n_slots + WGRP - 1) // WGRP
    return slot, max(n_groups, 1)


def _weave(blocks):
    """Interleave the four (ki, mi) chains, rotating quadrants."""
    chains = {(ki, mi): [] for ki in (0, 1) for mi in (0, 1)}
    for b in blocks:
        c, mi, w, ki = b
        chains[(ki, mi)].append(b)
    order = [(0, 0), (1, 1), (0, 1), (1, 0)]
    woven = []
    for q in chains:
        # moved (b-tile) blocks last so late b-tile arrival never gates
        # the front of a chain
        chains[q].sort(key=lambda b: (b[3] != b[0] % 2, b[0]))
    while any(chains.values()):
        for q in order:
            if chains[q]:
                woven.append(chains[q].pop(0))
    return woven


def _pack_host_arrays(weight, bias, pairs, slot, n_groups):
    wgrp = np.zeros((n_groups, 128, WGRP * BLK), dtype=BF16)
    bias_pk = np.zeros((128, len(pairs)), dtype=np.float32)
    wT = np.ascontiguousarray(
        np.transpose(np.asarray(weight), (0, 2, 1))).astype(BF16)
    for p, blocks in enumerate(pairs):
        r1, r2 = 2 * p, 2 * p + 1
        bias_pk[:64, p] = bias[r1 * BLK:(r1 + 1) * BLK]
        bias_pk[64:, p] = bias[r2 * BLK:(r2 + 1) * BLK]
        for (c, mi, w, ki) in blocks:
            g, j = divmod(slot[(p, c, mi)], WGRP)
            wgrp[g, ki * 64:(ki + 1) * 64, j * BLK:(j + 1) * BLK] = wT[w]
    return wgrp, bias_pk


def _build_kernel(pairs, slot, n_groups, btiles, ntok,
                  w_bufs=5, ps_bufs=8, out_bufs=6):
    assert ntok % 512 == 0
    n_th = ntok // 512
    sdt = mybir.dt.bfloat16
    f32 = mybir.dt.float32

    nc = bacc.Bacc("TRN2", target_bir_lowering=False, debug=False)
    n_chunks = (IN_BLK // 2 + XCH - 1) // XCH
    xt_d = nc.dram_tensor("xt_pk", [n_chunks, 128, XCH * ntok], sdt,
                          kind="ExternalInput").ap()
    n_b = max(len(btiles), 1)
    xb_d = nc.dram_tensor("xb_pk", [n_b, 128, ntok], sdt,
                          kind="ExternalInput").ap()
    wg_d = nc.dram_tensor("wgrp", [n_groups, 128, WGRP * BLK], sdt,
                          kind="ExternalInput").ap()
    bias_d = nc.dram_tensor("bias_pk", [128, len(pairs)], f32,
                            kind="ExternalInput").ap()
    yt_d = nc.dram_tensor("yt", [D_OUT, ntok], sdt,
                          kind="ExternalOutput").ap()
    b_index = {tb: k for k, tb in enumerate(btiles)}

    with tile.TileContext(nc) as tc:
        with ExitStack() as ctx:
            xpool = ctx.enter_context(tc.tile_pool(name="xp", bufs=1))
            wpool = ctx.enter_context(tc.tile_pool(name="wp", bufs=w_bufs))
            pspool = ctx.enter_context(
                tc.tile_pool(name="ps", bufs=ps_bufs, space="PSUM"))
            opool = ctx.enter_context(tc.tile_pool(name="op", bufs=out_bufs))
            bpool = ctx.enter_context(tc.tile_pool(name="bp", bufs=1))

            bias_sb = bpool.tile([128, len(pairs)], f32, tag="bias",
                                 name="bias_sb")
            nc.scalar.dma_start(bias_sb[:], bias_d[:])

            xchunks = {}
            xbt = {}

            def x_chunk(cb):
                if cb not in xchunks:
                    t = xpool.tile([128, XCH * ntok], sdt, tag=f"x{cb}",
                                   name=f"x{cb}")
                    nc.sync.dma_start(t[:], xt_d[cb, :, :])
                    xchunks[cb] = t
                return xchunks[cb]

            def x_btile(tb):
                if tb not in xbt:
                    t = xpool.tile([128, ntok], sdt, tag=f"xb{tb}",
                                   name=f"xb{tb}")
                    nc.sync.dma_start(t[:], xb_d[b_index[tb], :, :])
                    xbt[tb] = t
                return xbt[tb]

            def x_ap(c, ki, th):
                if ki == c % 2:
                    b = c // 2
                    cb, wi = divmod(b, XCH)
                    t = x_chunk(cb)
                    o = wi * ntok + th * 512
                else:
                    t = x_btile(_btile_of(c))
                    o = th * 512
                return t[ki * 64:(ki + 1) * 64, o:o + 512]

            # eager-issue all x chunks ahead of everything else
            for cb in range(n_chunks):
                x_chunk(cb)

            nmm = []
            for blocks in pairs:
                m = {(ki, mi): 0 for ki in (0, 1) for mi in (0, 1)}
                for (c, mi, w, ki) in blocks:
                    m[(ki, mi)] += 1
                nmm.append(m)

            psum = {}
            wg_tiles = {}

            def ensure_psum(p, th):
                if (p, th) not in psum:
                    psum[(p, th)] = [
                        pspool.tile([128, 512], f32, tag="ps",
                                    name=f"ps{p}_{th}_{k}") for k in range(2)]

            def ensure_wgroup(g):
                for gpf in (g, g + 1, g + 2):
                    if gpf < n_groups and gpf not in wg_tiles:
                        wg_tiles[gpf] = wpool.tile(
                            [128, WGRP * BLK], sdt, tag="wg",
                            name=f"wg{gpf}")
                        nc.scalar.dma_start(wg_tiles[gpf][:],
                                            wg_d[gpf, :, :])

            osb_cur = {}

            def eviction_th(p, th):
                if p not in osb_cur:
                    osb_cur[p] = opool.tile([128, n_th * 512], sdt,
                                            tag="o16", name=f"o{p}")
                osb = osb_cur[p][:, th * 512:(th + 1) * 512]
                pt = psum.pop((p, th))
                if all(v > 0 for v in nmm[p].values()):
                    nc.scalar.activation(
                        osb, pt[0][:],
                        mybir.ActivationFunctionType.Identity,
                        bias=bias_sb[:, p:p + 1], scale=1.0)
                    nc.vector.tensor_add(osb, osb, pt[1][:])
                else:
                    for mi in (0, 1):
                        oh = osb[mi * 64:(mi + 1) * 64, :]
                        bh = bias_sb[mi * 64:(mi + 1) * 64, p:p + 1]
                        srcs = [pt[ki][mi * 64:(mi + 1) * 64, :]
                                for ki in (0, 1) if nmm[p][(ki, mi)] > 0]
                        if not srcs:
                            nc.vector.memset(oh, 0.0)
                            nc.vector.tensor_scalar_add(oh, oh, bh)
                        else:
                            nc.scalar.activation(
                                oh, srcs[0],
                                mybir.ActivationFunctionType.Identity,
                                bias=bh, scale=1.0)
                            if len(srcs) > 1:
                                nc.vector.tensor_add(oh, oh, srcs[1])
                if th == n_th - 1:
                    nc.sync.dma_start(yt_d[p * 128:(p + 1) * 128, :],
                                      osb_cur.pop(p)[:])

            for p, blocks in enumerate(pairs):
                if not blocks:
                    osb = opool.tile([128, n_th * 512], sdt, tag="o16",
                                     name=f"oz{p}")
                    nc.vector.memset(osb[:], 0.0)
                    nc.vector.tensor_scalar_add(osb[:], osb[:],
                                                bias_sb[:, p:p + 1])
                    nc.sync.dma_start(yt_d[p * 128:(p + 1) * 128, :], osb[:])
                    continue
                for th in range(n_th):
                    ensure_psum(p, th)
                done = {(th, ki, mi): 0 for th in range(n_th)
                        for ki in (0, 1) for mi in (0, 1)}
                for (c, mi, w, ki) in _weave(blocks):
                    g, j = divmod(slot[(p, c, mi)], WGRP)
                    ensure_wgroup(g)
                    lhsT = wg_tiles[g][ki * 64:(ki + 1) * 64,
                                       j * BLK:(j + 1) * BLK]
                    for th in range(n_th):
                        done[(th, ki, mi)] += 1
                        first = done[(th, ki, mi)] == 1
                        last = done[(th, ki, mi)] == nmm[p][(ki, mi)]
                        nc.tensor.matmul(
                            psum[(p, th)][ki][mi * 64:(mi + 1) * 64, :],
                            lhsT, x_ap(c, ki, th),
                            start=first, stop=last,
                            tile_position=(ki * 64, mi * 64),
                            skip_group_check=True,
                        )
                for th in range(n_th):
                    eviction_th(p, th)
    nc.compile()
    return nc


def kernel(x, weight, bias, row_idx, col_idx):
    x = np.asarray(x, dtype=np.float32)
    weight = np.asarray(weight, dtype=np.float32)
    bias = np.asarray(bias, dtype=np.float32)
    row_idx = np.asarray(row_idx)
    col_idx = np.asarray(col_idx)
    ntok_total = x.shape[0]
    assert ntok_total % N_CORES == 0
    ntok = ntok_total // N_CORES

    pairs, btiles = _make_schedule(row_idx, col_idx)
    slot, n_groups = _assign_slots(pairs)
    wgrp, bias_pk = _pack_host_arrays(weight, bias, pairs, slot, n_groups)
    nc = _build_kernel(pairs, slot, n_groups, btiles, ntok)

    n_chunks = (IN_BLK // 2 + XCH - 1) // XCH
    in_maps = []
    for c in range(N_CORES):
        xt = np.ascontiguousarray(
            x[c * ntok:(c + 1) * ntok].T).astype(BF16)
        # chunk-contiguous: [cb, partition, wi*ntok + t]
        xt_pk = np.ascontiguousarray(
            xt.reshape(n_chunks, XCH, 128, ntok).transpose(0, 2, 1, 3)
            .reshape(n_chunks, 128, XCH * ntok))
        if btiles:
            xb_pk = np.stack(
                [xt[(2 * tb + 1) * 64:(2 * tb + 1) * 64 + 128] for tb in btiles])
        else:
            xb_pk = np.zeros((1, 128, ntok), dtype=BF16)
        xb_pk = np.ascontiguousarray(xb_pk)
        in_maps.append({"xt_pk": xt_pk, "xb_pk": xb_pk, "wgrp": wgrp,
                        "bias_pk": bias_pk})

    res = run_bass_kernel_spmd(nc, in_maps, core_ids=list(range(N_CORES)))
    y = np.empty((ntok_total, D_OUT), dtype=np.float32)
    for c in range(N_CORES):
        y[c * ntok:(c + 1) * ntok] = res.results[c]["yt"].T.astype(np.float32)
    return y
